# revision 41
# baseline (speedup 1.0000x reference)
"""Distributed Bass kernel for masked multi-head self-attention on 8 TRN2 NeuronCores.

Problem: x[2,2048,1024] -> qkv -> 16-head attention with outer-product mask
(keep[i,j] = mask[i]*mask[j]) -> out proj.  Masked queries produce exactly 0
rows and masked keys are fully excluded, so only the ~m unmasked tokens per
batch participate.  Host-side we compact tokens per batch, pad to a multiple
of 128 key slots (Kk), and split the batch dimension across the two 4-core
groups; within a group each core owns mq = Kk/4 query tokens.

Collectives on this part have a ~60-120us floor, far more than the ~25us of
extra TensorE time it takes to just recompute K and V for the whole batch on
every core of the group - so each core computes full-batch K^T and V locally
(x^T arrives host-pretransposed; no collective, no bounce buffers).

Padded token slots have x=0, so their keys score exp(0)=1 against every
query; the softmax denominator is corrected by subtracting the pad count
(a runtime input, keeping the device graph identical across cores).
Compute dtype is bf16 (f32 PSUM accumulation); softmax runs without
max-subtraction (scores are O(5), exp is safe in f32).
"""

import math
from contextlib import ExitStack

import numpy as np
import ml_dtypes

import concourse.bass as bass
import concourse.mybir as mybir
import concourse.tile as tile
from concourse import bacc
from concourse.bass_utils import run_bass_kernel_spmd
P = 128
HEADS = 16
DH = 64
D = 1024          # model dim
INNER = 1024      # heads * dh
SCALE = DH ** -0.5
N_CORES = 8
RPB = 4           # ranks (cores) per batch
BF16 = mybir.dt.bfloat16
F32 = mybir.dt.float32


def _build(mq: int, qe: int | None = None, dbg: bool = False):
    """Build the per-core SPMD graph for mq queries/core (mq % 32 == 0)."""
    qe = mq if qe is None else qe   # active query columns (<= mq)
    Kk = RPB * mq               # key slots per batch, multiple of 128
    VW = DH + 1                 # V~ cols per head
    nkt = Kk // P               # 128-row key tiles
    TT = math.ceil(qe / P)      # query-token tiles per core
    KCH = 384                   # K^T free-dim chunk (psum-bank friendly)
    nkch = math.ceil(Kk / KCH)

    def tspan(tt):
        return min(P, qe - tt * P)

    nc = bacc.Bacc(None, target_bir_lowering=False, num_devices=N_CORES)

    xt_in = nc.declare_dram_parameter("xt", [D, Kk], BF16, isOutput=False)
    xqt_in = nc.declare_dram_parameter("xqt", [D, qe], BF16, isOutput=False)
    wqkv_in = nc.declare_dram_parameter("wqkv", [D, 3 * INNER], BF16, isOutput=False)
    wout_in = nc.declare_dram_parameter("wout", [INNER, D], BF16, isOutput=False)
    npad_in = nc.declare_dram_parameter("npad", [1, 1], F32, isOutput=False)
    out_ext = nc.declare_dram_parameter("out", [qe, D], F32, isOutput=True)

    with tile.TileContext(nc) as tc, ExitStack() as ctx:
        sb = ctx.enter_context(tc.tile_pool(name="sb", bufs=1))
        ps = ctx.enter_context(tc.tile_pool(name="ps", bufs=1, space="PSUM"))

        npad_sb = sb.tile([1, 1], F32, tag="npad", bufs=1, name="npad_sb")
        nc.sync.dma_start(npad_sb[:], npad_in[:])

        # HAM warm-up: dependency-free matmuls on zeros so the PE clock is at
        # 2.4 GHz when the first real (DMA-gated) matmuls issue.
        warm = sb.tile([P, 512], BF16, tag="warm", bufs=1, name="warm")
        nc.vector.memset(warm[:], 0.0)
        wps = ps.tile([DH + 1, 512], F32, tag="av", bufs=2, name="wps")
        for i in range(9):
            nc.tensor.matmul(wps[:], warm[:, 0:DH + 1], warm[:],
                             start=True, stop=True)


        # ---- inputs: x^T (full batch + own query slice), weights.
        # Round-robin the DMA issues over four sequencers (issue costs ~600ns
        # serially per sequencer); wk+xt first, they gate the first matmul.
        seqs = [nc.sync, nc.scalar, nc.gpsimd]
        _n = [0]

        def dma(dst, src):
            seqs[_n[0] % len(seqs)].dma_start(dst, src)
            _n[0] += 1

        xt, xqt, wk, wv, wq = [], [], [], [], []
        for kc in range(8):
            tq_ = sb.tile([P, INNER], BF16, tag="wq", bufs=8, name=f"wq{kc}")
            dma(tq_[:, 0:512], wqkv_in[kc * P:(kc + 1) * P, 0:512])
            dma(tq_[:, 512:1024], wqkv_in[kc * P:(kc + 1) * P, 512:INNER])
            wq.append(tq_)
            tq = sb.tile([P, qe], BF16, tag="xqt", bufs=8, name=f"xqt{kc}")
            dma(tq[:], xqt_in[kc * P:(kc + 1) * P, :])
            xqt.append(tq)
        for kc in range(8):
            tk = sb.tile([P, INNER], BF16, tag="wk", bufs=8, name=f"wk{kc}")
            dma(tk[:, 0:512], wqkv_in[kc * P:(kc + 1) * P, INNER:INNER + 512])
            dma(tk[:, 512:1024], wqkv_in[kc * P:(kc + 1) * P, INNER + 512:2 * INNER])
            wk.append(tk)
            t_ = sb.tile([P, Kk], BF16, tag="xt", bufs=8, name=f"xt{kc}")
            dma(t_[:], xt_in[kc * P:(kc + 1) * P, :])
            xt.append(t_)
        for kc in range(8):
            tv = sb.tile([P, INNER], BF16, tag="wv", bufs=8, name=f"wv{kc}")
            dma(tv[:], wqkv_in[kc * P:(kc + 1) * P, 2 * INNER:3 * INNER])
            wv.append(tv)

        wout_sb = []
        for t in range(8):
            tw = sb.tile([P, D], BF16, tag="wout", bufs=8, name=f"wo{t}")
            dma(tw[:], wout_in[t * P:(t + 1) * P, :])
            wout_sb.append(tw)

        # ---- attention.  CH=2 staging groups = exactly one key-tile (both
        # heads), 3 staging slots, AV deferred one group so the PE never waits
        # on the exp.  Pair 0 is interleaved into the V phase above via
        # pair_step (its AV(kt) only needs vt[kt]); pairs 1-7 run after.
        CH = 2
        aoT = []
        pair_state = {}

        def pair_begin(hp):
            avp_a = ps.tile([P, qe], F32, tag="av", bufs=2, name=f"av{hp}a")
            avp_b = ps.tile([P, qe], F32, tag="av", bufs=2, name=f"av{hp}b")
            pair_state[hp] = {"avp": [avp_a, avp_b], "pending": None, "defer": []}

        def emit_av(hp, grp, pt_):
            avp = pair_state[hp]["avp"]
            for j, (kt, h) in enumerate(grp):
                nc.tensor.matmul(
                    avp[h][0:VW, :],
                    vt[kt][:, (2 * hp + h) * VW:(2 * hp + h) * VW + VW],
                    pt_[:, j * qe:(j + 1) * qe],
                    start=(kt == 0), stop=(kt == nkt - 1),
                    skip_group_check=True)

        def pair_step(hp, kt, defer=False):
            st = pair_state[hp]
            grp = [(kt, 0), (kt, 1)]
            sps = ps.tile([P, CH * 512], F32, tag="ss", bufs=3, name=f"sps{hp}_{kt}")
            for j, (kt_, h) in enumerate(grp):
                nc.tensor.matmul(sps[:, j * 512: j * 512 + qe],
                                 kf[hp][:, kt_ * P:(kt_ + 1) * P],
                                 qtz[2 * hp + h][:],
                                 start=True, stop=True)
            tag, bufs = ("pt1", 3 * nkt) if defer else ("pt", 6)
            pt_ = sb.tile([P, CH * qe], BF16, tag=tag, bufs=bufs, name=f"pt{hp}_{kt}")
            nc.scalar.activation(
                pt_[:].rearrange("p (u c) -> p u c", c=qe)[:, 0:len(grp), :],
                sps[:].rearrange("p (u c) -> p u c", c=512)[:, 0:len(grp), 0:qe],
                mybir.ActivationFunctionType.Exp, scale=SCALE)
            if defer:
                st["defer"].append((grp, pt_))
                return
            if st["pending"] is not None:
                emit_av(hp, *st["pending"])
            st["pending"] = (grp, pt_)

        def pair_end(hp):
            st = pair_state[hp]
            for d_ in st["defer"]:
                emit_av(hp, *d_)
            st["defer"] = []
            if st["pending"] is not None:
                emit_av(hp, *st["pending"])
            avp_a, avp_b = st["avp"]
            # softmax denominators ride in row 64; subtract the pad count,
            # reciprocal, broadcast across the 64 head dims, scale, pack.
            ao = sb.tile([P, qe], BF16, tag="aoT", bufs=8, name=f"aoT{hp}")
            for h, avp_h in ((1, avp_b), (0, avp_a)):
                av_ = sb.tile([DH + 1, qe], F32, tag="aos", bufs=6, name=f"aos{hp}_{h}")
                nc.vector.tensor_copy(av_[:], avp_h[0:DH + 1, :])
                den = sb.tile([1, qe], F32, tag="den", bufs=6, name=f"den{hp}_{h}")
                nc.vector.tensor_scalar(den[:], av_[DH:DH + 1, :], npad_sb[0:1, 0:1],
                                        None, op0=mybir.AluOpType.subtract)
                rec = sb.tile([1, qe], F32, tag="rec", bufs=6, name=f"rec{hp}_{h}")
                nc.vector.reciprocal_approx_fast(rec[:], den[:])
                fac = sb.tile([DH, qe], F32, tag="fac", bufs=6, name=f"fac{hp}_{h}")
                nc.gpsimd.partition_broadcast(fac[:], rec[:])
                if h == 0:
                    nc.vector.tensor_tensor(ao[0:DH, :], av_[0:DH, :], fac[:],
                                            op=mybir.AluOpType.mult)
                else:
                    tmpb = sb.tile([DH, qe], BF16, tag="tmpb", bufs=4,
                                   name=f"tmpb{hp}")
                    nc.vector.tensor_tensor(tmpb[:], av_[0:DH, :], fac[:],
                                            op=mybir.AluOpType.mult)
                    # partition shift 0:64 -> 64:128 needs a DMA, not DVE
                    nc.sync.dma_start(ao[DH:P, :], tmpb[:])
            aoT.append(ao)


        # ---- Q^T (own slice), zero-padded per head: qtz[h] has head h's 64
        # dims in their packed partition rows, zeros in the other 64, so S^T
        # contracts over the full 128 rows sharing one K^T lhsT per head pair.
        qtz = [None] * HEADS
        for t in range(8):
            qps = ps.tile([P, qe], F32, tag="ss", bufs=3, name=f"qps{t}")
            kcs = [(t + i) % 8 for i in range(8)]
            for i, kc in enumerate(kcs):
                nc.tensor.matmul(qps[:], wq[kc][:, t * P:(t + 1) * P], xqt[kc][:],
                                 start=(i == 0), stop=(i == 7))
            a = sb.tile([P, qe], BF16, tag="qtz", bufs=HEADS, name=f"qtz{2 * t}")
            nc.vector.memset(a[64:128, :], 0.0)
            nc.vector.tensor_copy(a[0:64, :], qps[0:64, :])
            qtz[2 * t] = a
            b = sb.tile([P, qe], BF16, tag="qtz", bufs=HEADS, name=f"qtz{2 * t + 1}")
            nc.vector.memset(b[0:64, :], 0.0)
            nc.vector.tensor_copy(b[64:128, :], qps[64:128, :])
            qtz[2 * t + 1] = b

        # ---- K^T for the whole batch: kf[t] [128 featdims, Kk keys] bf16
        defer_pairs = []
        kf = []
        for t in range(8):
            kft = sb.tile([P, Kk], BF16, tag="kf", bufs=8, name=f"kf{t}")
            for ch in range(nkch):
                w_ = min(KCH, Kk - ch * KCH)
                kps = ps.tile([P, KCH], F32, tag="ss", bufs=3, name=f"kps{t}_{ch}")
                kcs = [(t + ch + i) % 8 for i in range(8)]
                for i, kc in enumerate(kcs):
                    nc.tensor.matmul(kps[:, 0:w_], wk[kc][:, t * P:(t + 1) * P],
                                     xt[kc][:, ch * KCH: ch * KCH + w_],
                                     start=(i == 0), stop=(i == 7))
                nc.vector.tensor_copy(kft[:, ch * KCH: ch * KCH + w_], kps[:, 0:w_])
            kf.append(kft)

        # ---- V~ for the whole batch: vt[kt] [128 keys, 16*(64+1)] bf16 with a
        # ones column per head (softmax denominator rides row 64 of AV psum).
        vt = []
        pair0_started = False
        for kt in range(nkt):
            t_ = sb.tile([P, HEADS * VW], BF16, tag="vt", bufs=nkt, name=f"vt{kt}")
            nc.gpsimd.memset(t_[:, :], 0.0)
            nc.gpsimd.memset(
                t_[:].rearrange("p (h c) -> p h c", c=VW)[:, :, DH:DH + 1], 1.0)
            for nf in range(2):
                vps = ps.tile([P, 512], F32, tag="ss", bufs=3, name=f"vps{kt}_{nf}")
                kcs = [(kt + nf + i) % 8 for i in range(8)]
                for i, kc in enumerate(kcs):
                    nc.tensor.matmul(vps[:], xt[kc][:, kt * P:(kt + 1) * P],
                                     wv[kc][:, nf * 512:(nf + 1) * 512],
                                     start=(i == 0), stop=(i == 7))
                nc.vector.tensor_copy(
                    t_[:].rearrange("p (h c) -> p h c", c=VW)[:, nf * 8:(nf + 1) * 8, 0:DH],
                    vps[:].rearrange("p (h c) -> p h c", c=DH))
            vt.append(t_)
            if not pair0_started:
                pair_begin(0)
                pair0_started = True
            pair_step(0, kt)

        pair_end(0)
        for hp in defer_pairs:
            pair_end(hp)
        for hp in range(1, 8):
            if hp in defer_pairs:
                continue
            pair_begin(hp)
            for kt in range(nkt):
                pair_step(hp, kt)
            pair_end(hp)

        # ---- out projection.
        for mt in range(TT):
            pm = tspan(mt)
            osb = sb.tile([P, D], F32, tag="osb", bufs=3, name=f"osb{mt}")
            for nf in range(2):
                op_ps = ps.tile([P, 512], F32, tag="ss", bufs=3, name=f"op{mt}_{nf}")
                ts_ = [(mt * 2 + nf + j) % 6 for j in range(6)] + [6, 7]
                for i, t in enumerate(ts_):
                    nc.tensor.matmul(op_ps[0:pm, :], aoT[t][:, mt * P: mt * P + pm],
                                     wout_sb[t][:, nf * 512:(nf + 1) * 512],
                                     start=(i == 0), stop=(i == 7))
                nc.vector.tensor_copy(osb[0:pm, nf * 512:(nf + 1) * 512],
                                      op_ps[0:pm, :])
                nc.sync.dma_start(
                    out_ext[mt * P: mt * P + pm, nf * 512:(nf + 1) * 512],
                    osb[0:pm, nf * 512:(nf + 1) * 512])

    nc.compile()
    return nc


_GRAPH_CACHE: dict = {}


def _get_graph(mq: int, qe: int):
    if (mq, qe) not in _GRAPH_CACHE:
        _GRAPH_CACHE[(mq, qe)] = _build(mq, qe)
    return _GRAPH_CACHE[(mq, qe)]


def kernel(x, mask, W_qkv, W_out):
    x = np.asarray(x, dtype=np.float32)
    mask = np.asarray(mask, dtype=np.float32)
    W_qkv = np.asarray(W_qkv, dtype=np.float32)
    W_out = np.asarray(W_out, dtype=np.float32)
    b, n, d = x.shape
    assert (b, d) == (2, D) and W_qkv.shape == (D, 3 * INNER)

    idx = [np.nonzero(mask[i] > 0.5)[0] for i in range(b)]
    m = [len(ix) for ix in idx]
    mq = max(32, math.ceil(max(m) / RPB / 32) * 32)
    Kk = RPB * mq
    # real tokens spread evenly over the 4 cores of each batch group so every
    # core computes at most qe active query columns
    chunks = [np.array_split(ix, RPB) for ix in idx]
    qe = max(4, math.ceil(max(len(c) for cs in chunks for c in cs) / 4) * 4)

    nc = _get_graph(mq, qe)

    bf16 = ml_dtypes.bfloat16
    xg = np.zeros((b, RPB, mq, d), dtype=np.float32)
    for i in range(b):
        for r in range(RPB):
            xg[i, r, :len(chunks[i][r])] = x[i][chunks[i][r]]
    xgT = np.ascontiguousarray(
        xg.astype(bf16).reshape(b, Kk, d).transpose(0, 2, 1))  # [b, D, Kk]
    wqkv_bf = W_qkv.astype(bf16)
    wout_bf = W_out.astype(bf16)

    in_maps = []
    for core in range(N_CORES):
        bi, r = divmod(core, RPB)
        in_maps.append({
            "xt": xgT[bi],
            "xqt": np.ascontiguousarray(xgT[bi][:, r * mq: r * mq + qe]),
            "wqkv": wqkv_bf,
            "wout": wout_bf,
            "npad": np.array([[Kk - m[bi]]], dtype=np.float32),
        })

    res = run_bass_kernel_spmd(nc, in_maps, core_ids=list(range(N_CORES)))

    out = np.zeros((b, n, d), dtype=np.float32)
    for bi in range(b):
        for r in range(RPB):
            ch = chunks[bi][r]
            out[bi][ch] = res.results[bi * RPB + r]["out"][:len(ch)]
    return out


# revision 42
# speedup vs baseline: 1.0177x; 1.0177x over previous
"""Distributed Bass kernel for masked multi-head self-attention on 8 TRN2 NeuronCores.

Problem: x[2,2048,1024] -> qkv -> 16-head attention with outer-product mask
(keep[i,j] = mask[i]*mask[j]) -> out proj.  Masked queries produce exactly 0
rows and masked keys are fully excluded, so only the ~m unmasked tokens per
batch participate.  Host-side we compact tokens per batch, pad to a multiple
of 128 key slots (Kk), and split the batch dimension across the two 4-core
groups; within a group each core owns mq = Kk/4 query tokens.

Collectives on this part have a ~60-120us floor, far more than the ~25us of
extra TensorE time it takes to just recompute K and V for the whole batch on
every core of the group - so each core computes full-batch K^T and V locally
(x^T arrives host-pretransposed; no collective, no bounce buffers).

Padded token slots have x=0, so their keys score exp(0)=1 against every
query; the softmax denominator is corrected by subtracting the pad count
(a runtime input, keeping the device graph identical across cores).
Compute dtype is bf16 (f32 PSUM accumulation); softmax runs without
max-subtraction (scores are O(5), exp is safe in f32).
"""

import math
from contextlib import ExitStack

import numpy as np
import ml_dtypes

import concourse.bass as bass
import concourse.mybir as mybir
import concourse.tile as tile
from concourse import bacc
from concourse.bass_utils import run_bass_kernel_spmd
P = 128
HEADS = 16
DH = 64
D = 1024          # model dim
INNER = 1024      # heads * dh
SCALE = DH ** -0.5
N_CORES = 8
RPB = 4           # ranks (cores) per batch
BF16 = mybir.dt.bfloat16
F32 = mybir.dt.float32


def _build(mq: int, qe: int | None = None, dbg: bool = False):
    """Build the per-core SPMD graph for mq queries/core (mq % 32 == 0)."""
    qe = mq if qe is None else qe   # active query columns (<= mq)
    Kk = RPB * mq               # key slots per batch, multiple of 128
    VW = DH + 1                 # V~ cols per head
    nkt = Kk // P               # 128-row key tiles
    TT = math.ceil(qe / P)      # query-token tiles per core
    KCH = 384                   # K^T free-dim chunk (psum-bank friendly)
    nkch = math.ceil(Kk / KCH)

    def tspan(tt):
        return min(P, qe - tt * P)

    nc = bacc.Bacc(None, target_bir_lowering=False, num_devices=N_CORES)

    xt_in = nc.declare_dram_parameter("xt", [D, Kk], BF16, isOutput=False)
    xqt_in = nc.declare_dram_parameter("xqt", [D, qe], BF16, isOutput=False)
    wqkv_in = nc.declare_dram_parameter("wqkv", [D, 3 * INNER], BF16, isOutput=False)
    wout_in = nc.declare_dram_parameter("wout", [INNER, D], BF16, isOutput=False)
    npad_in = nc.declare_dram_parameter("npad", [1, 1], F32, isOutput=False)
    out_ext = nc.declare_dram_parameter("out", [qe, D], F32, isOutput=True)

    with tile.TileContext(nc) as tc, ExitStack() as ctx:
        sb = ctx.enter_context(tc.tile_pool(name="sb", bufs=1))
        ps = ctx.enter_context(tc.tile_pool(name="ps", bufs=1, space="PSUM"))

        npad_sb = sb.tile([1, 1], F32, tag="npad", bufs=1, name="npad_sb")
        nc.sync.dma_start(npad_sb[:], npad_in[:])

        # HAM warm-up: dependency-free matmuls on zeros so the PE clock is at
        # 2.4 GHz when the first real (DMA-gated) matmuls issue.
        warm = sb.tile([P, 512], BF16, tag="warm", bufs=1, name="warm")
        nc.vector.memset(warm[:], 0.0)
        wps = ps.tile([DH + 1, 512], F32, tag="av", bufs=2, name="wps")
        for i in range(9):
            nc.tensor.matmul(wps[:], warm[:, 0:DH + 1], warm[:],
                             start=True, stop=True)


        # ---- inputs: x^T (full batch + own query slice), weights.
        # Round-robin the DMA issues over four sequencers (issue costs ~600ns
        # serially per sequencer); wk+xt first, they gate the first matmul.
        seqs = [nc.sync, nc.scalar, nc.gpsimd]
        _n = [0]

        def dma(dst, src):
            seqs[_n[0] % len(seqs)].dma_start(dst, src)
            _n[0] += 1

        xt, xqt, wk, wv, wq = [], [], [], [], []
        for kc in range(8):
            tq_ = sb.tile([P, INNER], BF16, tag="wq", bufs=8, name=f"wq{kc}")
            dma(tq_[:, 0:512], wqkv_in[kc * P:(kc + 1) * P, 0:512])
            dma(tq_[:, 512:1024], wqkv_in[kc * P:(kc + 1) * P, 512:INNER])
            wq.append(tq_)
            tq = sb.tile([P, qe], BF16, tag="xqt", bufs=8, name=f"xqt{kc}")
            dma(tq[:], xqt_in[kc * P:(kc + 1) * P, :])
            xqt.append(tq)
        for kc in range(8):
            tk = sb.tile([P, INNER], BF16, tag="wk", bufs=8, name=f"wk{kc}")
            dma(tk[:, 0:512], wqkv_in[kc * P:(kc + 1) * P, INNER:INNER + 512])
            dma(tk[:, 512:1024], wqkv_in[kc * P:(kc + 1) * P, INNER + 512:2 * INNER])
            wk.append(tk)
            t_ = sb.tile([P, Kk], BF16, tag="xt", bufs=8, name=f"xt{kc}")
            dma(t_[:], xt_in[kc * P:(kc + 1) * P, :])
            xt.append(t_)
        for kc in range(8):
            tv = sb.tile([P, INNER], BF16, tag="wv", bufs=8, name=f"wv{kc}")
            dma(tv[:], wqkv_in[kc * P:(kc + 1) * P, 2 * INNER:3 * INNER])
            wv.append(tv)

        wout_sb = []
        for t in range(8):
            tw = sb.tile([P, D], BF16, tag="wout", bufs=8, name=f"wo{t}")
            dma(tw[:], wout_in[t * P:(t + 1) * P, :])
            wout_sb.append(tw)

        # ---- attention.  CH=2 staging groups = exactly one key-tile (both
        # heads), 3 staging slots, AV deferred one group so the PE never waits
        # on the exp.  Pair 0 is interleaved into the V phase above via
        # pair_step (its AV(kt) only needs vt[kt]); pairs 1-7 run after.
        CH = 2
        aoT = []
        pair_state = {}

        def pair_begin(hp):
            avp_a = ps.tile([P, qe], F32, tag="av", bufs=2, name=f"av{hp}a")
            avp_b = ps.tile([P, qe], F32, tag="av", bufs=2, name=f"av{hp}b")
            pair_state[hp] = {"avp": [avp_a, avp_b], "pending": None, "defer": []}

        def emit_av(hp, grp, pt_):
            avp = pair_state[hp]["avp"]
            for j, (kt, h) in enumerate(grp):
                nc.tensor.matmul(
                    avp[h][0:VW, :],
                    vt[kt][:, (2 * hp + h) * VW:(2 * hp + h) * VW + VW],
                    pt_[:, j * qe:(j + 1) * qe],
                    start=(kt == 0), stop=(kt == nkt - 1),
                    skip_group_check=True)

        def pair_step(hp, kt, defer=False):
            st = pair_state[hp]
            grp = [(kt, 0), (kt, 1)]
            sps = ps.tile([P, CH * 512], F32, tag="ss", bufs=3, name=f"sps{hp}_{kt}")
            for j, (kt_, h) in enumerate(grp):
                nc.tensor.matmul(sps[:, j * 512: j * 512 + qe],
                                 kf[hp][:, kt_ * P:(kt_ + 1) * P],
                                 qtz[2 * hp + h][:],
                                 start=True, stop=True)
            tag, bufs = ("pt1", 3 * nkt) if defer else ("pt", 6)
            pt_ = sb.tile([P, CH * qe], BF16, tag=tag, bufs=bufs, name=f"pt{hp}_{kt}")
            nc.scalar.activation(
                pt_[:].rearrange("p (u c) -> p u c", c=qe)[:, 0:len(grp), :],
                sps[:].rearrange("p (u c) -> p u c", c=512)[:, 0:len(grp), 0:qe],
                mybir.ActivationFunctionType.Exp, scale=SCALE)
            if defer:
                st["defer"].append((grp, pt_))
                return
            if st["pending"] is not None:
                emit_av(hp, *st["pending"])
            st["pending"] = (grp, pt_)

        def pair_end(hp):
            st = pair_state[hp]
            for d_ in st["defer"]:
                emit_av(hp, *d_)
            st["defer"] = []
            if st["pending"] is not None:
                emit_av(hp, *st["pending"])
            avp_a, avp_b = st["avp"]
            # softmax denominators ride in row 64; subtract the pad count,
            # reciprocal, broadcast across the 64 head dims, scale, pack.
            ao = sb.tile([P, qe], BF16, tag="aoT", bufs=8, name=f"aoT{hp}")
            for h, avp_h in enumerate((avp_a, avp_b)):
                av_ = sb.tile([DH + 1, qe], F32, tag="aos", bufs=6, name=f"aos{hp}_{h}")
                nc.vector.tensor_copy(av_[:], avp_h[0:DH + 1, :])
                den = sb.tile([1, qe], F32, tag="den", bufs=6, name=f"den{hp}_{h}")
                nc.vector.tensor_scalar(den[:], av_[DH:DH + 1, :], npad_sb[0:1, 0:1],
                                        None, op0=mybir.AluOpType.subtract)
                rec = sb.tile([1, qe], F32, tag="rec", bufs=6, name=f"rec{hp}_{h}")
                nc.vector.reciprocal_approx_fast(rec[:], den[:])
                fac = sb.tile([DH, qe], F32, tag="fac", bufs=6, name=f"fac{hp}_{h}")
                nc.gpsimd.partition_broadcast(fac[:], rec[:])
                if h == 0:
                    nc.vector.tensor_tensor(ao[0:DH, :], av_[0:DH, :], fac[:],
                                            op=mybir.AluOpType.mult)
                else:
                    tmpb = sb.tile([DH, qe], BF16, tag="tmpb", bufs=4,
                                   name=f"tmpb{hp}")
                    nc.vector.tensor_tensor(tmpb[:], av_[0:DH, :], fac[:],
                                            op=mybir.AluOpType.mult)
                    # partition shift 0:64 -> 64:128 needs a DMA, not DVE
                    nc.sync.dma_start(ao[DH:P, :], tmpb[:])
            aoT.append(ao)


        # ---- Q^T (own slice), zero-padded per head: qtz[h] has head h's 64
        # dims in their packed partition rows, zeros in the other 64, so S^T
        # contracts over the full 128 rows sharing one K^T lhsT per head pair.
        qtz = [None] * HEADS
        for t in range(8):
            qps = ps.tile([P, qe], F32, tag="ss", bufs=3, name=f"qps{t}")
            kcs = [(t + i) % 8 for i in range(8)]
            for i, kc in enumerate(kcs):
                nc.tensor.matmul(qps[:], wq[kc][:, t * P:(t + 1) * P], xqt[kc][:],
                                 start=(i == 0), stop=(i == 7))
            a = sb.tile([P, qe], BF16, tag="qtz", bufs=HEADS, name=f"qtz{2 * t}")
            nc.vector.memset(a[64:128, :], 0.0)
            nc.vector.tensor_copy(a[0:64, :], qps[0:64, :])
            qtz[2 * t] = a
            b = sb.tile([P, qe], BF16, tag="qtz", bufs=HEADS, name=f"qtz{2 * t + 1}")
            nc.vector.memset(b[0:64, :], 0.0)
            nc.vector.tensor_copy(b[64:128, :], qps[64:128, :])
            qtz[2 * t + 1] = b

        # ---- K^T for the whole batch: kf[t] [128 featdims, Kk keys] bf16
        defer_pairs = []
        kf = []
        for t in range(8):
            kft = sb.tile([P, Kk], BF16, tag="kf", bufs=8, name=f"kf{t}")
            for ch in range(nkch):
                w_ = min(KCH, Kk - ch * KCH)
                kps = ps.tile([P, KCH], F32, tag="ss", bufs=3, name=f"kps{t}_{ch}")
                kcs = [(t + ch + i) % 8 for i in range(8)]
                for i, kc in enumerate(kcs):
                    nc.tensor.matmul(kps[:, 0:w_], wk[kc][:, t * P:(t + 1) * P],
                                     xt[kc][:, ch * KCH: ch * KCH + w_],
                                     start=(i == 0), stop=(i == 7))
                nc.vector.tensor_copy(kft[:, ch * KCH: ch * KCH + w_], kps[:, 0:w_])
            kf.append(kft)

        # ---- V~ for the whole batch: vt[kt] [128 keys, 16*(64+1)] bf16 with a
        # ones column per head (softmax denominator rides row 64 of AV psum).
        vt = []
        pair0_started = False
        for kt in range(nkt):
            t_ = sb.tile([P, HEADS * VW], BF16, tag="vt", bufs=nkt, name=f"vt{kt}")
            nc.gpsimd.memset(t_[:, :], 0.0)
            nc.gpsimd.memset(
                t_[:].rearrange("p (h c) -> p h c", c=VW)[:, :, DH:DH + 1], 1.0)
            for nf in range(2):
                vps = ps.tile([P, 512], F32, tag="ss", bufs=3, name=f"vps{kt}_{nf}")
                kcs = [(kt + nf + i) % 8 for i in range(8)]
                for i, kc in enumerate(kcs):
                    nc.tensor.matmul(vps[:], xt[kc][:, kt * P:(kt + 1) * P],
                                     wv[kc][:, nf * 512:(nf + 1) * 512],
                                     start=(i == 0), stop=(i == 7))
                nc.vector.tensor_copy(
                    t_[:].rearrange("p (h c) -> p h c", c=VW)[:, nf * 8:(nf + 1) * 8, 0:DH],
                    vps[:].rearrange("p (h c) -> p h c", c=DH))
            vt.append(t_)
            if not pair0_started:
                pair_begin(0)
                pair0_started = True
            pair_step(0, kt)

        pair_end(0)
        for hp in defer_pairs:
            pair_end(hp)
        for hp in range(1, 8):
            if hp in defer_pairs:
                continue
            pair_begin(hp)
            for kt in range(nkt):
                pair_step(hp, kt)
            pair_end(hp)

        # ---- out projection.
        for mt in range(TT):
            pm = tspan(mt)
            osb = sb.tile([P, D], F32, tag="osb", bufs=3, name=f"osb{mt}")
            for nf in range(2):
                op_ps = ps.tile([P, 512], F32, tag="ss", bufs=3, name=f"op{mt}_{nf}")
                ts_ = [(mt * 2 + nf + j) % 8 for j in range(8)]
                for i, t in enumerate(ts_):
                    nc.tensor.matmul(op_ps[0:pm, :], aoT[t][:, mt * P: mt * P + pm],
                                     wout_sb[t][:, nf * 512:(nf + 1) * 512],
                                     start=(i == 0), stop=(i == 7))
                nc.vector.tensor_copy(osb[0:pm, nf * 512:(nf + 1) * 512],
                                      op_ps[0:pm, :])
                nc.sync.dma_start(
                    out_ext[mt * P: mt * P + pm, nf * 512:(nf + 1) * 512],
                    osb[0:pm, nf * 512:(nf + 1) * 512])

    nc.compile()
    return nc


_GRAPH_CACHE: dict = {}


def _get_graph(mq: int, qe: int):
    if (mq, qe) not in _GRAPH_CACHE:
        _GRAPH_CACHE[(mq, qe)] = _build(mq, qe)
    return _GRAPH_CACHE[(mq, qe)]


def kernel(x, mask, W_qkv, W_out):
    x = np.asarray(x, dtype=np.float32)
    mask = np.asarray(mask, dtype=np.float32)
    W_qkv = np.asarray(W_qkv, dtype=np.float32)
    W_out = np.asarray(W_out, dtype=np.float32)
    b, n, d = x.shape
    assert (b, d) == (2, D) and W_qkv.shape == (D, 3 * INNER)

    idx = [np.nonzero(mask[i] > 0.5)[0] for i in range(b)]
    m = [len(ix) for ix in idx]
    mq = max(32, math.ceil(max(m) / RPB / 32) * 32)
    Kk = RPB * mq
    # real tokens spread evenly over the 4 cores of each batch group so every
    # core computes at most qe active query columns
    chunks = [np.array_split(ix, RPB) for ix in idx]
    qe = max(4, math.ceil(max(len(c) for cs in chunks for c in cs) / 4) * 4)

    nc = _get_graph(mq, qe)

    bf16 = ml_dtypes.bfloat16
    xg = np.zeros((b, RPB, mq, d), dtype=np.float32)
    for i in range(b):
        for r in range(RPB):
            xg[i, r, :len(chunks[i][r])] = x[i][chunks[i][r]]
    xgT = np.ascontiguousarray(
        xg.astype(bf16).reshape(b, Kk, d).transpose(0, 2, 1))  # [b, D, Kk]
    wqkv_bf = W_qkv.astype(bf16)
    wout_bf = W_out.astype(bf16)

    in_maps = []
    for core in range(N_CORES):
        bi, r = divmod(core, RPB)
        in_maps.append({
            "xt": xgT[bi],
            "xqt": np.ascontiguousarray(xgT[bi][:, r * mq: r * mq + qe]),
            "wqkv": wqkv_bf,
            "wout": wout_bf,
            "npad": np.array([[Kk - m[bi]]], dtype=np.float32),
        })

    res = run_bass_kernel_spmd(nc, in_maps, core_ids=list(range(N_CORES)))

    out = np.zeros((b, n, d), dtype=np.float32)
    for bi in range(b):
        for r in range(RPB):
            ch = chunks[bi][r]
            out[bi][ch] = res.results[bi * RPB + r]["out"][:len(ch)]
    return out
